# revision 1
# baseline (speedup 1.0000x reference)
"""Multi-class contrastive loss on 8 Trainium2 NeuronCores.

Math (reference):
  e = emb / max(||emb||, 1e-12)                      (row-normalize)
  centers = segment_mean(e, labels, C);  cs = centers / max(||centers||, 1e-8)
  sims = e @ cs.T                                    [N, C]
  pos_i = (sims[i, l_i] - 1)^2
  neg_i = (sum_c relu(1-sims)^2 - relu(1-sims[i,l_i])^2) / (C-1)
  loss = mean(pos + neg)

Key facts exploited:
  * All sims are cosine similarities of unit-norm vectors => sims <= 1, so
    relu(1 - s)^2 == (1 - s)^2 everywhere, and pos_i == relu(1-s_label)^2.
  * Therefore loss = [A + (C-2)*B] / (C-1) / N with
      A = sum_{i,c} (1 - sims)^2        (the only O(N*C*D) term)
      B = sum_i (1 - s_label_i)^2       (O(N*D), done on host)
  * Per-class segment sums / centers are O(N*D) and done on host
    (sort + reduceat), so the device kernel is label-free and fully static.

Device kernel (per core, rows sharded 8 x 8192):
  sims chunk [128 rows, <=512 classes] via 2 bf16 matmuls (d = 2x128 contraction),
  then one ScalarE ACTIVATE(Square, scale=-1, bias=1, accum_out) per chunk
  accumulates sum_c (1-s)^2 per row. A single reduce at the end emits 128
  partial sums per core; the host combines them with B.
"""

import functools
from contextlib import ExitStack

import numpy as np
import ml_dtypes

N_TOTAL = 65536
D = 256
C = 1000
NCORES = 8
ROWS = N_TOTAL // NCORES          # 8192 rows per core
BLOCKS = ROWS // 128              # 64 row blocks per core
CPAD = 1024                       # class dim padded for tidy layout
CHUNKS = ((0, 512), (512, 488))   # class chunks (cover exactly 0..999)
NORM_EPS = 1e-12
COS_EPS = 1e-8

BF16 = ml_dtypes.bfloat16


@functools.lru_cache(maxsize=1)
def _build_module():
    import concourse.tile as tile
    from concourse import bacc, mybir

    nc = bacc.Bacc("TRN2", target_bir_lowering=False, debug=False)
    et_d = nc.dram_tensor(
        "et_in", [128, BLOCKS, 2, 128], mybir.dt.bfloat16, kind="ExternalInput"
    )
    cst_d = nc.dram_tensor(
        "cst_in", [128, 2, CPAD], mybir.dt.bfloat16, kind="ExternalInput"
    )
    out_d = nc.dram_tensor(
        "partials_out", [128, 1], mybir.dt.float32, kind="ExternalOutput"
    )

    with tile.TileContext(nc) as tc:
        with ExitStack() as ctx:
            const_pool = ctx.enter_context(tc.tile_pool(name="const", bufs=1))
            et_pool = ctx.enter_context(tc.tile_pool(name="et", bufs=4))
            ps_pool = ctx.enter_context(tc.tile_pool(name="ps", bufs=4, space="PSUM"))
            scr_pool = ctx.enter_context(tc.tile_pool(name="scr", bufs=3))
            acc_pool = ctx.enter_context(tc.tile_pool(name="acc", bufs=1))

            cst = const_pool.tile([128, 2, CPAD], mybir.dt.bfloat16)
            nc.sync.dma_start(cst[:], cst_d.ap()[:])

            accA = acc_pool.tile([128, 2 * BLOCKS], mybir.dt.float32)

            for b in range(BLOCKS):
                et = et_pool.tile([128, 2, 128], mybir.dt.bfloat16)
                nc.sync.dma_start(et[:], et_d.ap()[:, b, :, :])
                for ci, (c0, cw) in enumerate(CHUNKS):
                    ps = ps_pool.tile([128, 512], mybir.dt.float32)
                    nc.tensor.matmul(
                        ps[:, 0:cw], et[:, 0, :], cst[:, 0, c0 : c0 + cw],
                        start=True, stop=False,
                    )
                    nc.tensor.matmul(
                        ps[:, 0:cw], et[:, 1, :], cst[:, 1, c0 : c0 + cw],
                        start=False, stop=True,
                    )
                    scr = scr_pool.tile([128, 512], mybir.dt.bfloat16)
                    col = 2 * b + ci
                    nc.scalar.activation(
                        scr[:, 0:cw], ps[:, 0:cw],
                        mybir.ActivationFunctionType.Square,
                        bias=1.0, scale=-1.0,
                        accum_out=accA[:, col : col + 1],
                    )

            total = acc_pool.tile([128, 1], mybir.dt.float32)
            nc.vector.tensor_reduce(
                total[:], accA[:], axis=mybir.AxisListType.X, op=mybir.AluOpType.add
            )
            nc.sync.dma_start(out_d.ap()[:], total[:])

    nc.compile()
    return nc


def _prep(embeddings, labels):
    """Host-side O(N*D) pipeline: normalize, centers, B-term, device layouts."""
    emb = np.ascontiguousarray(np.asarray(embeddings, dtype=np.float32))
    lab = np.asarray(labels).astype(np.int64).ravel()
    n = emb.shape[0]

    nrm = np.sqrt(np.einsum("nd,nd->n", emb, emb, dtype=np.float64))
    nrm = np.maximum(nrm, NORM_EPS).astype(np.float32)
    e_n = emb / nrm[:, None]                          # [N, D] fp32, unit rows

    counts = np.bincount(lab, minlength=C)
    order = np.argsort(lab, kind="stable")
    lab_sorted = lab[order]
    e_sorted = e_n[order]
    starts = np.searchsorted(lab_sorted, np.arange(C))
    idx = np.minimum(starts, n - 1)
    sums = np.add.reduceat(e_sorted, idx, axis=0)     # [C, D]
    sums[counts == 0] = 0.0
    centers = sums / np.maximum(counts, 1)[:, None].astype(np.float32)
    cn = np.sqrt(np.einsum("cd,cd->c", centers, centers, dtype=np.float64))
    denom = np.maximum(cn, COS_EPS)
    cs = (centers / denom[:, None]).astype(np.float32)  # [C, D] scaled centers

    # B = sum_i (1 - e_i . cs[l_i])^2  in float64
    s_lab = np.einsum("nd,nd->n", e_n, cs[lab])
    B_tot = float(np.sum((1.0 - s_lab) ** 2, dtype=np.float64))

    # Device layouts (bf16)
    e_nb = e_n.astype(BF16)
    et_list = []
    for c in range(NCORES):
        shard = e_nb[c * ROWS : (c + 1) * ROWS]        # [8192, 256]
        # ET[p, b, f, j] = shard[b*128 + j, f*128 + p]
        et = shard.reshape(BLOCKS, 128, 2, 128).transpose(3, 0, 2, 1)
        et_list.append(np.ascontiguousarray(et))

    cs_pad = np.zeros((CPAD, D), dtype=np.float32)
    cs_pad[:C] = cs
    # CST[p, f, c] = cs_pad[c, f*128 + p]
    cst = np.ascontiguousarray(cs_pad.astype(BF16).reshape(CPAD, 2, 128).transpose(2, 1, 0))

    return et_list, cst, B_tot


def _make_in_maps(et_list, cst):
    return [{"et_in": et_list[c], "cst_in": cst} for c in range(NCORES)]


def _run_device(in_maps, trace=False):
    from concourse import bass_utils

    nc = _build_module()
    return bass_utils.run_bass_kernel_spmd(
        nc, in_maps, core_ids=list(range(NCORES)), trace=trace
    )


def _combine(results, B_tot):
    A_tot = 0.0
    for r in results:
        A_tot += float(np.asarray(r["partials_out"], dtype=np.float64).sum())
    loss = (A_tot + (C - 2) * B_tot) / (C - 1) / N_TOTAL
    return np.float32(loss)


def kernel(embeddings, labels):
    et_list, cst, B_tot = _prep(embeddings, labels)
    res = _run_device(_make_in_maps(et_list, cst))
    return _combine(res.results, B_tot)
